# revision 17
# baseline (speedup 1.0000x reference)
"""Trainium2 Bass kernel for MultiHeadedAttention with learned per-key-position scaling.

Sharding over 8 NeuronCores: batch(2) x head-group(4).  Each core owns one
batch's full 2048-query / 2048-key sequence and 3 of the 12 heads (a 192-wide
d_model slice of the QKV/output projections).  There are NO collectives: the
per-(batch, key-position) scaling divisor delta depends only on the query
rows, which every core holds in full, so each core computes delta locally.

Scores are computed transposed ([kpos, q]) so that:
  - the per-key-position divisor delta folds into the exp's per-partition scale,
  - the softmax denominator Z comes from a ones-column appended to V,
  - P@V needs no on-chip transposition of the attention probabilities.

Precision: activations and weights are fp16 (host-cast; ~2.4e-4 quantization,
negligible against the bf16 attention probabilities), matmuls accumulate in
fp32 PSUM, attention probabilities are bf16 (fp16 would overflow: the
unnormalized exp reaches e^35).  fp16 moving operands run the PE at full rate
and halve the HBM traffic of the fp32 baseline.

Host sums the 4 per-core partial outputs of each batch (head-group pieces of
the output projection) and adds bo.
"""

import sys

for _p in ("/opt/trn_rl_repo",):
    if _p not in sys.path:
        sys.path.insert(0, _p)

import numpy as np
import ml_dtypes

BF16 = ml_dtypes.bfloat16

B, S, D, H, DK = 2, 2048, 768, 12, 64
NCORES = 8
HH = H // 4          # heads per core
DH = HH * DK         # 192 head dims per core

_cache = {}


def _build(s=S, hh=HH, d=D, dk=DK):
    import concourse.bass as bass
    import concourse.mybir as mybir
    import concourse.tile as tile
    from concourse import bacc

    f32 = mybir.dt.float32
    f16 = mybir.dt.float16
    bf = mybir.dt.bfloat16
    Exp = mybir.ActivationFunctionType.Exp
    mult = mybir.AluOpType.mult
    add = mybir.AluOpType.add
    amin = mybir.AluOpType.min
    amax = mybir.AluOpType.max

    dh = hh * dk         # 192
    KC = s // 128        # 16 key-position chunks
    C6 = d // 128        # 6 d_model chunks
    SQH = s // 2         # 1024-wide q half (attention unit width)
    BW = 256             # streaming block width (projection inputs)
    NB = s // BW         # 8 stream blocks for q/k/v
    KCL = BW // 128      # 2 kpos chunks per stream block

    nc = bacc.Bacc("TRN2", target_bir_lowering=False, debug=False, num_devices=NCORES)

    t = {}
    t["qT"] = nc.dram_tensor("qT", [d, s], f16, kind="ExternalInput").ap()
    t["kT"] = nc.dram_tensor("kT", [d, s], f16, kind="ExternalInput").ap()
    t["vT"] = nc.dram_tensor("vT", [d, s], f16, kind="ExternalInput").ap()
    t["maskT"] = nc.dram_tensor("maskT", [s, s], bf, kind="ExternalInput").ap()
    t["wq"] = nc.dram_tensor("wq", [d, dh], f16, kind="ExternalInput").ap()
    t["wk"] = nc.dram_tensor("wk", [d, dh], f16, kind="ExternalInput").ap()
    t["wv"] = nc.dram_tensor("wv", [d, dh], f16, kind="ExternalInput").ap()
    t["wo"] = nc.dram_tensor("wo", [dh, d], f16, kind="ExternalInput").ap()
    t["wd"] = nc.dram_tensor("wd", [d, 1], f16, kind="ExternalInput").ap()
    t["bq"] = nc.dram_tensor("bq", [dh], f32, kind="ExternalInput").ap()
    t["bk"] = nc.dram_tensor("bk", [dh], f32, kind="ExternalInput").ap()
    t["bv"] = nc.dram_tensor("bv", [dh], f32, kind="ExternalInput").ap()
    t["bd"] = nc.dram_tensor("bd", [1], f32, kind="ExternalInput").ap()
    t["yp"] = nc.dram_tensor("yp", [s, d], f16, kind="ExternalOutput").ap()

    # [d, *] tensors viewed as [128, C6, *] partition tiles
    def dview(ap):
        return ap.rearrange("(c p) s -> p c s", p=128)

    def bcast(ap, n):
        # broadcast a 1-D DRAM vector across n partitions
        return bass.AP(tensor=ap.tensor, offset=ap.offset, ap=[[0, n]] + list(ap.ap))

    with tile.TileContext(nc) as tc:
        with (
            tc.tile_pool(name="persist", bufs=1) as P,
            tc.tile_pool(name="pj", bufs=2, space="PSUM") as PJ,
            tc.tile_pool(name="xpp", bufs=2, space="PSUM") as XPP,
            tc.tile_pool(name="work", bufs=4) as W,
            tc.tile_pool(name="work2", bufs=2) as W2,
            tc.tile_pool(name="load", bufs=4) as L,
            tc.tile_pool(name="loadv", bufs=2) as LF,
            tc.tile_pool(name="dram", bufs=2, space="DRAM") as DR,
        ):
            maskTs = P.tile([128, KC, s], bf)
            vsb = P.tile([128, KC, hh, dk + 1], f16)
            # heads 0,1 packed on partitions; head 2 on a 64-partition tile
            qa = P.tile([128, s], f16)
            qb_ = P.tile([64, s], f16)
            ka = P.tile([128, s], f16)
            kb_ = P.tile([64, s], f16)
            xa = P.tile([128, s], f16)
            xb_ = P.tile([64, s], f16)
            # wq chunk 1 (64 cols) is stored with the wd column appended so a
            # single 65-wide stationary computes head-2 q features AND the
            # delta row at zero extra PE cost
            wqd_sb = P.tile([128, C6, dh + 1], f16)
            wk_sb = P.tile([128, C6, dh], f16)
            wv_sb = P.tile([128, C6, dh], f16)
            wo_a = P.tile([128, d], f16)
            wo_b = P.tile([64, d], f16)
            bqa = P.tile([128, 1], f32)
            bqb = P.tile([64, 1], f32)
            bka = P.tile([128, 1], f32)
            bkb = P.tile([64, 1], f32)
            bvb = P.tile([128, hh, dk], f32)
            bdb = P.tile([128, 1], f32)
            rdcol = P.tile([128, KC], f32)
            dstage = P.tile([65, s], f32)

            # PE warm-up spin: the p-state model runs matmuls 2-4x slower
            # until the PE has been continuously busy for 3us.  Burn the ramp
            # on throwaway matmuls so the real pipeline starts at full clock.
            wtile = W.tile([128, 512], f16, tag="warm")
            nc.vector.memset(wtile, 0.0)
            wps = XPP.tile([128, 512], f32, tag="xps")
            for _ in range(8):
                nc.tensor.matmul(wps, lhsT=wtile[:, 0:128], rhs=wtile, start=True, stop=True)

            # warm the ACT exp table while DMAs stream
            dummy = W.tile([1, 2], f32, tag="dummy")
            nc.vector.memset(dummy, 0.0)
            nc.scalar.activation(dummy, dummy, Exp, scale=1.0)

            nc.gpsimd.dma_start(bdb, bcast(t["bd"], 128))
            nc.sync.dma_start(bqa, t["bq"][0:128].rearrange("(c p) -> p c", p=128))
            nc.sync.dma_start(bqb, t["bq"][128:dh].rearrange("(c p) -> p c", p=64))
            nc.vector.memset(vsb[:, :, :, dk : dk + 1], 1.0)

            def stream_blk(name, blk, tag="ldf", pool=L, dt=f16):
                fr = pool.tile([128, C6, BW], dt, tag=tag, name=f"{name}_{blk}")
                nc.sync.dma_start(fr, dview(t[name])[:, :, blk * BW : (blk + 1) * BW])
                return fr

            def q_block(blk):
                qqb = stream_blk("qT", blk)
                if blk == 0:
                    # q weights ride behind the first query block
                    nc.sync.dma_start(wqd_sb[:, :, 0:dh], dview(t["wq"]))
                    nc.sync.dma_start(wqd_sb[:, :, dh : dh + 1], dview(t["wd"]))
                qp = PJ.tile([128, BW], f32, tag="pj")
                for c in range(C6):
                    nc.tensor.matmul(
                        qp,
                        lhsT=wqd_sb[:, c, 0:128],
                        rhs=qqb[:, c, :],
                        start=(c == 0),
                        stop=(c == C6 - 1),
                    )
                nc.vector.tensor_scalar_add(
                    out=qa[:, blk * BW : (blk + 1) * BW], in0=qp, scalar1=bqa
                )
                qp1 = PJ.tile([65, BW], f32, tag="pj")
                for c in range(C6):
                    nc.tensor.matmul(
                        qp1[0:65, :],
                        lhsT=wqd_sb[:, c, 128 : dh + 1],
                        rhs=qqb[:, c, :],
                        start=(c == 0),
                        stop=(c == C6 - 1),
                    )
                nc.vector.tensor_scalar_add(
                    out=qb_[0:64, blk * BW : (blk + 1) * BW],
                    in0=qp1[0:64, :],
                    scalar1=bqb[0:64, :],
                )
                nc.vector.tensor_copy(
                    dstage[64:65, blk * BW : (blk + 1) * BW], qp1[64:65, :]
                )

            def delta_stage(half):
                # delta rows for q blocks [4*half, 4*half+4) -> rdcol[:, 8h:8h+8]
                # DRAM roundtrip re-partitions [1, 1024] -> [128, 8]
                # on the ACT queue: the sync queue is busy with MB-scale
                # stream loads and would head-of-line block this
                # latency-critical 4KB roundtrip; ACT is idle at both points
                # where delta stages run
                lo = half * (s // 2)
                ddr = DR.tile([s // 2], f32, name=f"ddr{half}")
                nc.scalar.dma_start(ddr, dstage[64:65, lo : lo + s // 2])
                dloc = W2.tile([128, KC // 2], f32, tag="dloc")
                nc.scalar.dma_start(dloc, ddr.rearrange("(kc p) -> p kc", p=128))
                nc.vector.tensor_scalar(
                    out=dloc, in0=dloc, scalar1=bdb, scalar2=0.0, op0=add, op1=amax
                )
                nc.vector.tensor_scalar(
                    out=dloc, in0=dloc, scalar1=8.0, scalar2=1.0, op0=amin, op1=add
                )
                nc.vector.reciprocal(
                    rdcol[:, half * (KC // 2) : (half + 1) * (KC // 2)], dloc
                )

            # stream emitters, interleaved with unit-0 attention below
            def k_block(blk):
                kfb = stream_blk("kT", blk)
                kp = PJ.tile([128, BW], f32, tag="pj")
                for c in range(C6):
                    nc.tensor.matmul(
                        kp,
                        lhsT=wk_sb[:, c, 0:128],
                        rhs=kfb[:, c, :],
                        start=(c == 0),
                        stop=(c == C6 - 1),
                    )
                nc.vector.tensor_scalar_add(
                    out=ka[:, blk * BW : (blk + 1) * BW], in0=kp, scalar1=bka
                )
                kp1 = PJ.tile([128, BW], f32, tag="pj")
                for c in range(C6):
                    nc.tensor.matmul(
                        kp1[0:64, :],
                        lhsT=wk_sb[:, c, 128:dh],
                        rhs=kfb[:, c, :],
                        start=(c == 0),
                        stop=(c == C6 - 1),
                    )
                nc.vector.tensor_scalar_add(
                    out=kb_[0:64, blk * BW : (blk + 1) * BW],
                    in0=kp1[0:64, :],
                    scalar1=bkb[0:64, :],
                )

            def v_block(blk):
                vfb = stream_blk("vT", blk, tag="vb", pool=LF)
                for kcl in range(KCL):
                    kc = blk * KCL + kcl
                    vp = PJ.tile([128, dh], f32, tag="pj")
                    for c in range(C6):
                        nc.tensor.matmul(
                            vp,
                            lhsT=vfb[:, c, kcl * 128 : (kcl + 1) * 128],
                            rhs=wv_sb[:, c, :],
                            start=(c == 0),
                            stop=(c == C6 - 1),
                        )
                    nc.vector.tensor_tensor(
                        out=vsb[:, kc, :, 0:dk],
                        in0=vp.rearrange("p (h e) -> p h e", h=hh),
                        in1=bvb,
                        op=add,
                    )

            mview = t["maskT"].rearrange("(kc p) q -> p kc q", p=128)

            def mask_g(qh, g):
                # 2 kpos-chunks x one q-half per transfer (0.5 MB)
                nc.gpsimd.dma_start(
                    maskTs[:, 2 * g : 2 * g + 2, qh * SQH : (qh + 1) * SQH],
                    mview[:, 2 * g : 2 * g + 2, qh * SQH : (qh + 1) * SQH],
                )

            # --- pre-attention stream: just enough q/k/v for unit 0's start ---
            mask_g(0, 0)
            q_block(0)
            q_block(1)
            q_block(2)
            q_block(3)
            nc.sync.dma_start(wk_sb, dview(t["wk"]))
            nc.sync.dma_start(bka, t["bk"][0:128].rearrange("(c p) -> p c", p=128))
            nc.sync.dma_start(bkb, t["bk"][128:dh].rearrange("(c p) -> p c", p=64))
            k_block(0)
            delta_stage(0)
            nc.sync.dma_start(wv_sb, dview(t["wv"]))
            nc.gpsimd.dma_start(
                bvb, bcast(t["bv"].rearrange("(h e) -> h e", h=hh), 128)
            )
            v_block(0)

            # JIT emission schedule for the remaining stream blocks inside
            # unit 0: k_block(j)/v_block(j) produce kpos chunks {2j, 2j+1},
            # q_block(4+i) feeds qh1 and the delta rows for kpos 1024+
            UNIT0_EMIT = {
                0: [("k", 1)],
                1: [("v", 1), ("q", 4)],
                2: [("k", 2), ("q", 5)],
                3: [("v", 2), ("q", 6)],
                4: [("k", 3), ("q", 7)],
                5: [("v", 3), ("d", 1)],
                6: [("k", 4), ("v", 4)],
                7: [("v", 5)],
                8: [("k", 5)],
                9: [("v", 6)],
                10: [("k", 6)],
                11: [("v", 7)],
                12: [("k", 7)],
            }
            m_done = [1, 0]      # mask groups issued per q-half

            def out_proj(qc):
                yps = PJ.tile([128, d], f32, tag="pj")
                for col in range(0, d, 512):
                    ncol = min(512, d - col)
                    nc.tensor.matmul(
                        yps[:, col : col + ncol],
                        lhsT=xa[:, qc * 128 : (qc + 1) * 128],
                        rhs=wo_a[:, col : col + ncol],
                        start=True,
                        stop=False,
                    )
                    nc.tensor.matmul(
                        yps[:, col : col + ncol],
                        lhsT=xb_[0:64, qc * 128 : (qc + 1) * 128],
                        rhs=wo_b[0:64, col : col + ncol],
                        start=False,
                        stop=True,
                    )
                ysb = W2.tile([128, d], f16, tag="ysb", bufs=4)
                if qc % 2 == 0:
                    nc.scalar.copy(ysb, yps)
                else:
                    nc.vector.tensor_copy(ysb, yps)
                nc.sync.dma_start(t["yp"][qc * 128 : (qc + 1) * 128, :], ysb)

            for unit in range(2 * hh):
                qh, h = unit // hh, unit % hh
                hoff = (h % 2) * 64
                if h < 2:
                    ksl, qsl = ka, qa
                else:
                    ksl, qsl = kb_, qb_
                qcol0 = qh * SQH
                xps = XPP.tile([dk + 1, SQH], f32, tag="xps")
                for kc in range(KC):
                    sps = PJ.tile([128, SQH], f32, tag="pj")
                    for nn in range(SQH // 512):
                        nc.tensor.matmul(
                            sps[:, nn * 512 : (nn + 1) * 512],
                            lhsT=ksl[hoff : hoff + 64, kc * 128 : (kc + 1) * 128],
                            rhs=qsl[
                                hoff : hoff + 64,
                                qcol0 + nn * 512 : qcol0 + (nn + 1) * 512,
                            ],
                            start=True,
                            stop=True,
                        )
                    psb = W.tile([128, SQH], bf, tag="psb")
                    nc.scalar.activation(psb, sps, Exp, scale=rdcol[:, kc : kc + 1])
                    nc.vector.tensor_tensor(
                        out=psb,
                        in0=psb,
                        in1=maskTs[:, kc, qcol0 : qcol0 + SQH],
                        op=mult,
                    )
                    for nn in range(SQH // 512):
                        nc.tensor.matmul(
                            xps[:, nn * 512 : (nn + 1) * 512],
                            lhsT=vsb[:, kc, h, :],
                            rhs=psb[:, nn * 512 : (nn + 1) * 512],
                            start=(kc == 0),
                            stop=(kc == KC - 1),
                        )
                    # stream emission AFTER the attention ops so the in-order
                    # PE runs scores/PV ahead of stream matmuls awaiting DMA
                    if unit == 0:
                        for item in UNIT0_EMIT.get(kc, ()):
                            if item[0] == "k":
                                k_block(item[1])
                            elif item[0] == "v":
                                v_block(item[1])
                            elif item[0] == "q":
                                q_block(item[1])
                            else:
                                delta_stage(item[1])
                        # qh0 mask kept ahead of consumption
                        while m_done[0] < KC // 2 and m_done[0] <= (kc + KCL) // 2:
                            mask_g(0, m_done[0])
                            m_done[0] += 1
                        if kc == KC - 1:
                            nc.sync.dma_start(wo_a, t["wo"][0:128, :])
                            nc.sync.dma_start(wo_b, t["wo"][128:dh, :])
                    elif unit in (1, 2) and kc % 4 == 3:
                        # qh1 mask spread across units 1-2
                        if m_done[1] < KC // 2:
                            mask_g(1, m_done[1])
                            m_done[1] += 1

                rz = W2.tile([1, SQH], f32, tag="rz", bufs=1)
                nc.vector.reciprocal(rz, xps[dk : dk + 1, :])
                zdr = DR.tile([SQH], f32)
                nc.sync.dma_start(zdr, rz)
                rzb = W2.tile([64, SQH], f32, tag="rzb", bufs=1)
                nc.gpsimd.dma_start(rzb, bcast(zdr, 64))
                xsl = xa if h < 2 else xb_
                nc.vector.tensor_tensor(
                    out=xsl[hoff : hoff + 64, qcol0 : qcol0 + SQH],
                    in0=xps[0:dk, :],
                    in1=rzb,
                    op=mult,
                )
                while m_done[1] < KC // 2:
                    mask_g(1, m_done[1])
                    m_done[1] += 1

            # --- output projection ---
            for qc in range(S // 128):
                out_proj(qc)

    nc.compile()
    return nc


def _in_maps(query, key, value, mask, Wq, bq, Wk, bk, Wv, bv, Wo, Wd, bd):
    query = np.asarray(query, np.float32)
    key = np.asarray(key, np.float32)
    value = np.asarray(value, np.float32)
    mask = np.asarray(mask)
    qT = [np.ascontiguousarray(query[b].T).astype(np.float16) for b in range(B)]
    kT = [np.ascontiguousarray(key[b].T).astype(np.float16) for b in range(B)]
    vT = [np.ascontiguousarray(value[b].T).astype(np.float16) for b in range(B)]
    mT = [np.ascontiguousarray(mask[b].T).astype(BF16) for b in range(B)]
    wqh = np.ascontiguousarray(Wq).astype(np.float16)
    wkh = np.ascontiguousarray(Wk).astype(np.float16)
    wvh = np.ascontiguousarray(Wv).astype(np.float16)
    woh = np.ascontiguousarray(Wo).astype(np.float16)
    wdh = np.ascontiguousarray(Wd).astype(np.float16)
    bqf = np.ascontiguousarray(bq, np.float32)
    bkf = np.ascontiguousarray(bk, np.float32)
    bvf = np.ascontiguousarray(bv, np.float32)
    bdf = np.ascontiguousarray(bd, np.float32)

    maps = []
    for c in range(NCORES):
        b, hg = c // 4, c % 4
        hs = slice(hg * DH, (hg + 1) * DH)
        maps.append(
            {
                "qT": qT[b],
                "kT": kT[b],
                "vT": vT[b],
                "maskT": mT[b],
                "wq": np.ascontiguousarray(wqh[:, hs]),
                "wk": np.ascontiguousarray(wkh[:, hs]),
                "wv": np.ascontiguousarray(wvh[:, hs]),
                "wo": np.ascontiguousarray(woh[hs, :]),
                "wd": wdh,
                "bq": np.ascontiguousarray(bqf[hs]),
                "bk": np.ascontiguousarray(bkf[hs]),
                "bv": np.ascontiguousarray(bvf[hs]),
                "bd": bdf,
            }
        )
    return maps


def kernel(query, key, value, mask, Wq, bq, Wk, bk, Wv, bv, Wo, bo, Wd, bd):
    from concourse.bass_utils import run_bass_kernel_spmd

    if "nc" not in _cache:
        _cache["nc"] = _build()
    nc = _cache["nc"]

    maps = _in_maps(query, key, value, mask, Wq, bq, Wk, bk, Wv, bv, Wo, Wd, bd)
    res = run_bass_kernel_spmd(nc, maps, core_ids=list(range(NCORES)))

    bof = np.asarray(bo, np.float32)
    y = np.empty((B, S, D), np.float32)
    for b in range(B):
        acc = res.results[b * 4]["yp"].astype(np.float32)
        for hg in range(1, 4):
            acc = acc + res.results[b * 4 + hg]["yp"].astype(np.float32)
        y[b] = acc + bof[None, :]
    return y


# revision 21
# speedup vs baseline: 1.1218x; 1.1218x over previous
"""Trainium2 Bass kernel for MultiHeadedAttention with learned per-key-position scaling.

Sharding over 8 NeuronCores: batch(2) x head-group(4).  Each core owns one
batch's full 2048-query / 2048-key sequence and 3 of the 12 heads (a 192-wide
d_model slice of the QKV/output projections).  There are NO collectives: the
per-(batch, key-position) scaling divisor delta depends only on the query
rows, which every core holds in full, so each core computes delta locally.

Scores are computed transposed ([kpos, q]) so that:
  - the per-key-position divisor delta folds into the exp's per-partition scale,
  - the softmax denominator Z comes from a ones-column appended to V,
  - P@V needs no on-chip transposition of the attention probabilities.

Precision: activations and weights are fp16 (host-cast; ~2.4e-4 quantization,
negligible against the bf16 attention probabilities), matmuls accumulate in
fp32 PSUM, attention probabilities are bf16 (fp16 would overflow: the
unnormalized exp reaches e^35).  fp16 moving operands run the PE at full rate
and halve the HBM traffic of the fp32 baseline.

Host sums the 4 per-core partial outputs of each batch (head-group pieces of
the output projection) and adds bo.
"""

import sys

for _p in ("/opt/trn_rl_repo",):
    if _p not in sys.path:
        sys.path.insert(0, _p)

import numpy as np
import ml_dtypes

BF16 = ml_dtypes.bfloat16

B, S, D, H, DK = 2, 2048, 768, 12, 64
NCORES = 8
HH = H // 4          # heads per core
DH = HH * DK         # 192 head dims per core

_cache = {}


def _build(s=S, hh=HH, d=D, dk=DK):
    import concourse.bass as bass
    import concourse.mybir as mybir
    import concourse.tile as tile
    from concourse import bacc

    f32 = mybir.dt.float32
    f16 = mybir.dt.float16
    bf = mybir.dt.bfloat16
    Exp = mybir.ActivationFunctionType.Exp
    mult = mybir.AluOpType.mult
    add = mybir.AluOpType.add
    amin = mybir.AluOpType.min
    amax = mybir.AluOpType.max

    dh = hh * dk         # 192
    KC = s // 128        # 16 key-position chunks
    C6 = d // 128        # 6 d_model chunks
    SQH = s // 2         # 1024-wide q half (attention unit width)
    BW = 256             # streaming block width (projection inputs)
    NB = s // BW         # 8 stream blocks for q/k/v
    KCL = BW // 128      # 2 kpos chunks per stream block

    nc = bacc.Bacc("TRN2", target_bir_lowering=False, debug=False, num_devices=NCORES)

    t = {}
    t["qT"] = nc.dram_tensor("qT", [d, s], f16, kind="ExternalInput").ap()
    t["kT"] = nc.dram_tensor("kT", [d, s], f16, kind="ExternalInput").ap()
    t["vT"] = nc.dram_tensor("vT", [d, s], f16, kind="ExternalInput").ap()
    t["maskT"] = nc.dram_tensor("maskT", [s, s], bf, kind="ExternalInput").ap()
    t["wq"] = nc.dram_tensor("wq", [d, dh], f16, kind="ExternalInput").ap()
    t["wk"] = nc.dram_tensor("wk", [d, dh], f16, kind="ExternalInput").ap()
    t["wv"] = nc.dram_tensor("wv", [d, dh], f16, kind="ExternalInput").ap()
    t["wo"] = nc.dram_tensor("wo", [dh, d], f16, kind="ExternalInput").ap()
    t["wd"] = nc.dram_tensor("wd", [d, 1], f16, kind="ExternalInput").ap()
    t["bq"] = nc.dram_tensor("bq", [dh], f32, kind="ExternalInput").ap()
    t["bk"] = nc.dram_tensor("bk", [dh], f32, kind="ExternalInput").ap()
    t["bv"] = nc.dram_tensor("bv", [dh], f32, kind="ExternalInput").ap()
    t["bd"] = nc.dram_tensor("bd", [1], f32, kind="ExternalInput").ap()
    t["yp"] = nc.dram_tensor("yp", [s, d], f16, kind="ExternalOutput").ap()

    # [d, *] tensors viewed as [128, C6, *] partition tiles
    def dview(ap):
        return ap.rearrange("(c p) s -> p c s", p=128)

    def bcast(ap, n):
        # broadcast a 1-D DRAM vector across n partitions
        return bass.AP(tensor=ap.tensor, offset=ap.offset, ap=[[0, n]] + list(ap.ap))

    with tile.TileContext(nc) as tc:
        with (
            tc.tile_pool(name="persist", bufs=1) as P,
            tc.tile_pool(name="pj", bufs=2, space="PSUM") as PJ,
            tc.tile_pool(name="xpp", bufs=2, space="PSUM") as XPP,
            tc.tile_pool(name="work", bufs=4) as W,
            tc.tile_pool(name="work2", bufs=2) as W2,
            tc.tile_pool(name="load", bufs=4) as L,
            tc.tile_pool(name="loadv", bufs=2) as LF,
            tc.tile_pool(name="dram", bufs=2, space="DRAM") as DR,
        ):
            maskTs = P.tile([128, KC, s], bf)
            vsb = P.tile([128, KC, hh, dk + 1], f16)
            # heads 0,1 packed on partitions; head 2 on a 64-partition tile
            qa = P.tile([128, s], f16)
            qb_ = P.tile([64, s], f16)
            ka = P.tile([128, s], f16)
            kb_ = P.tile([64, s], f16)
            xa = P.tile([128, s], f16)
            xb_ = P.tile([64, s], f16)
            # wq chunk 1 (64 cols) is stored with the wd column appended so a
            # single 65-wide stationary computes head-2 q features AND the
            # delta row at zero extra PE cost
            wqd_sb = P.tile([128, C6, dh + 1], f16)
            wk_sb = P.tile([128, C6, dh], f16)
            wv_sb = P.tile([128, C6, dh], f16)
            wo_a = P.tile([128, d], f16)
            wo_b = P.tile([64, d], f16)
            bqa = P.tile([128, 1], f32)
            bqb = P.tile([64, 1], f32)
            bka = P.tile([128, 1], f32)
            bkb = P.tile([64, 1], f32)
            bvb = P.tile([128, hh, dk], f32)
            bdb = P.tile([128, 1], f32)
            rdcol = P.tile([128, KC], f32)
            dstage = P.tile([65, s], f32)

            # PE warm-up spin: the p-state model runs matmuls 2-4x slower
            # until the PE has been continuously busy for 3us.  Burn the ramp
            # on throwaway matmuls so the real pipeline starts at full clock.
            wtile = W.tile([128, 512], f16, tag="warm")
            nc.vector.memset(wtile, 0.0)
            wps = XPP.tile([128, 512], f32, tag="xps")
            for _ in range(8):
                nc.tensor.matmul(wps, lhsT=wtile[:, 0:128], rhs=wtile, start=True, stop=True)

            # warm the ACT exp table while DMAs stream
            dummy = W.tile([1, 2], f32, tag="dummy")
            nc.vector.memset(dummy, 0.0)
            nc.scalar.activation(dummy, dummy, Exp, scale=1.0)

            nc.gpsimd.dma_start(bdb, bcast(t["bd"], 128))
            nc.vector.memset(vsb[:, :, :, dk : dk + 1], 1.0)

            def stream_blk(name, blk, tag="ldf", pool=L, dt=f16):
                fr = pool.tile([128, C6, BW], dt, tag=tag, name=f"{name}_{blk}")
                nc.sync.dma_start(fr, dview(t[name])[:, :, blk * BW : (blk + 1) * BW])
                return fr

            def q_block(blk):
                qqb = stream_blk("qT", blk)
                if blk == 0:
                    # q weights ride behind the first query block
                    nc.sync.dma_start(wqd_sb[:, :, 0:dh], dview(t["wq"]))
                    nc.sync.dma_start(wqd_sb[:, :, dh : dh + 1], dview(t["wd"]))
                qp = PJ.tile([128, BW], f32, tag="pj")
                for c in range(C6):
                    nc.tensor.matmul(
                        qp,
                        lhsT=wqd_sb[:, c, 0:128],
                        rhs=qqb[:, c, :],
                        start=(c == 0),
                        stop=(c == C6 - 1),
                    )
                nc.vector.tensor_scalar_add(
                    out=qa[:, blk * BW : (blk + 1) * BW], in0=qp, scalar1=bqa
                )
                qp1 = PJ.tile([65, BW], f32, tag="pj")
                for c in range(C6):
                    nc.tensor.matmul(
                        qp1[0:65, :],
                        lhsT=wqd_sb[:, c, 128 : dh + 1],
                        rhs=qqb[:, c, :],
                        start=(c == 0),
                        stop=(c == C6 - 1),
                    )
                nc.vector.tensor_scalar_add(
                    out=qb_[0:64, blk * BW : (blk + 1) * BW],
                    in0=qp1[0:64, :],
                    scalar1=bqb[0:64, :],
                )
                nc.vector.tensor_copy(
                    dstage[64:65, blk * BW : (blk + 1) * BW], qp1[64:65, :]
                )

            def delta_stage(half):
                # delta rows for q blocks [4*half, 4*half+4) -> rdcol[:, 8h:8h+8]
                # DRAM roundtrip re-partitions [1, 1024] -> [128, 8]
                # on the ACT queue: the sync queue is busy with MB-scale
                # stream loads and would head-of-line block this
                # latency-critical 4KB roundtrip; ACT is idle at both points
                # where delta stages run
                lo = half * (s // 2)
                ddr = DR.tile([s // 2], f32, name=f"ddr{half}")
                nc.scalar.dma_start(ddr, dstage[64:65, lo : lo + s // 2])
                dloc = W2.tile([128, KC // 2], f32, tag="dloc")
                nc.scalar.dma_start(dloc, ddr.rearrange("(kc p) -> p kc", p=128))
                nc.vector.tensor_scalar(
                    out=dloc, in0=dloc, scalar1=bdb, scalar2=0.0, op0=add, op1=amax
                )
                nc.vector.tensor_scalar(
                    out=dloc, in0=dloc, scalar1=8.0, scalar2=1.0, op0=amin, op1=add
                )
                nc.vector.reciprocal(
                    rdcol[:, half * (KC // 2) : (half + 1) * (KC // 2)], dloc
                )

            # stream emitters, interleaved with unit-0 attention below
            def k_block(blk):
                kfb = stream_blk("kT", blk)
                kp = PJ.tile([128, BW], f32, tag="pj")
                for c in range(C6):
                    nc.tensor.matmul(
                        kp,
                        lhsT=wk_sb[:, c, 0:128],
                        rhs=kfb[:, c, :],
                        start=(c == 0),
                        stop=(c == C6 - 1),
                    )
                nc.vector.tensor_scalar_add(
                    out=ka[:, blk * BW : (blk + 1) * BW], in0=kp, scalar1=bka
                )
                kp1 = PJ.tile([128, BW], f32, tag="pj")
                for c in range(C6):
                    nc.tensor.matmul(
                        kp1[0:64, :],
                        lhsT=wk_sb[:, c, 128:dh],
                        rhs=kfb[:, c, :],
                        start=(c == 0),
                        stop=(c == C6 - 1),
                    )
                nc.vector.tensor_scalar_add(
                    out=kb_[0:64, blk * BW : (blk + 1) * BW],
                    in0=kp1[0:64, :],
                    scalar1=bkb[0:64, :],
                )

            def v_block(blk):
                vfb = stream_blk("vT", blk, tag="vb", pool=LF)
                for kcl in range(KCL):
                    kc = blk * KCL + kcl
                    vp = PJ.tile([128, dh], f32, tag="pj")
                    for c in range(C6):
                        nc.tensor.matmul(
                            vp,
                            lhsT=vfb[:, c, kcl * 128 : (kcl + 1) * 128],
                            rhs=wv_sb[:, c, :],
                            start=(c == 0),
                            stop=(c == C6 - 1),
                        )
                    nc.vector.tensor_tensor(
                        out=vsb[:, kc, :, 0:dk],
                        in0=vp.rearrange("p (h e) -> p h e", h=hh),
                        in1=bvb,
                        op=add,
                    )

            mview = t["maskT"].rearrange("(kc p) q -> p kc q", p=128)

            def mask_g(qh, g):
                # 2 kpos-chunks x one q-half per transfer (0.5 MB).  On the
                # sync queue deliberately: FIFO order between the stream
                # loads paces the bus; on an idle queue all groups fire at
                # once and starve the latency-critical k/v/q loads.
                nc.sync.dma_start(
                    maskTs[:, 2 * g : 2 * g + 2, qh * SQH : (qh + 1) * SQH],
                    mview[:, 2 * g : 2 * g + 2, qh * SQH : (qh + 1) * SQH],
                )

            # --- pre-attention stream: just enough q/k/v for unit 0's start.
            # biases ride the ACT queue so the sync queue leads with q0+wq
            nc.scalar.dma_start(bqa, t["bq"][0:128].rearrange("(c p) -> p c", p=128))
            nc.scalar.dma_start(bqb, t["bq"][128:dh].rearrange("(c p) -> p c", p=64))
            nc.scalar.dma_start(bka, t["bk"][0:128].rearrange("(c p) -> p c", p=128))
            nc.scalar.dma_start(bkb, t["bk"][128:dh].rearrange("(c p) -> p c", p=64))
            q_block(0)
            q_block(1)
            mask_g(0, 0)
            q_block(2)
            q_block(3)
            nc.sync.dma_start(wk_sb, dview(t["wk"]))
            k_block(0)
            delta_stage(0)
            nc.sync.dma_start(wv_sb, dview(t["wv"]))
            nc.gpsimd.dma_start(
                bvb, bcast(t["bv"].rearrange("(h e) -> h e", h=hh), 128)
            )
            v_block(0)

            # JIT emission schedule for the remaining stream blocks inside
            # unit 0: k_block(j)/v_block(j) produce kpos chunks {2j, 2j+1},
            # q_block(4+i) feeds qh1 and the delta rows for kpos 1024+
            UNIT0_EMIT = {
                0: [("k", 1)],
                1: [("v", 1), ("q", 4)],
                2: [("k", 2), ("q", 5)],
                3: [("v", 2), ("q", 6)],
                4: [("k", 3), ("q", 7)],
                5: [("v", 3), ("d", 1)],
                6: [("k", 4), ("v", 4)],
                7: [("v", 5)],
                8: [("k", 5)],
                9: [("v", 6)],
                10: [("k", 6)],
                11: [("v", 7)],
                12: [("k", 7)],
            }
            m_done = [1, 0]      # mask groups issued per q-half

            def out_proj(qc):
                yps = PJ.tile([128, d], f32, tag="pj")
                for col in range(0, d, 512):
                    ncol = min(512, d - col)
                    nc.tensor.matmul(
                        yps[:, col : col + ncol],
                        lhsT=xa[:, qc * 128 : (qc + 1) * 128],
                        rhs=wo_a[:, col : col + ncol],
                        start=True,
                        stop=False,
                    )
                    nc.tensor.matmul(
                        yps[:, col : col + ncol],
                        lhsT=xb_[0:64, qc * 128 : (qc + 1) * 128],
                        rhs=wo_b[0:64, col : col + ncol],
                        start=False,
                        stop=True,
                    )
                ysb = W2.tile([128, d], f16, tag="ysb", bufs=4)
                if qc % 2 == 0:
                    nc.scalar.copy(ysb, yps)
                else:
                    nc.vector.tensor_copy(ysb, yps)
                nc.sync.dma_start(t["yp"][qc * 128 : (qc + 1) * 128, :], ysb)

            for unit in range(2 * hh):
                qh, h = unit // hh, unit % hh
                hoff = (h % 2) * 64
                if h < 2:
                    ksl, qsl = ka, qa
                else:
                    ksl, qsl = kb_, qb_
                qcol0 = qh * SQH
                xps = XPP.tile([dk + 1, SQH], f32, tag="xps")
                for kc in range(KC):
                    sps = PJ.tile([128, SQH], f32, tag="pj")
                    for nn in range(SQH // 512):
                        nc.tensor.matmul(
                            sps[:, nn * 512 : (nn + 1) * 512],
                            lhsT=ksl[hoff : hoff + 64, kc * 128 : (kc + 1) * 128],
                            rhs=qsl[
                                hoff : hoff + 64,
                                qcol0 + nn * 512 : qcol0 + (nn + 1) * 512,
                            ],
                            start=True,
                            stop=True,
                        )
                    psb = W.tile([128, SQH], bf, tag="psb")
                    nc.scalar.activation(psb, sps, Exp, scale=rdcol[:, kc : kc + 1])
                    nc.vector.tensor_tensor(
                        out=psb,
                        in0=psb,
                        in1=maskTs[:, kc, qcol0 : qcol0 + SQH],
                        op=mult,
                    )
                    for nn in range(SQH // 512):
                        nc.tensor.matmul(
                            xps[:, nn * 512 : (nn + 1) * 512],
                            lhsT=vsb[:, kc, h, :],
                            rhs=psb[:, nn * 512 : (nn + 1) * 512],
                            start=(kc == 0),
                            stop=(kc == KC - 1),
                        )
                    # stream emission AFTER the attention ops so the in-order
                    # PE runs scores/PV ahead of stream matmuls awaiting DMA
                    if unit == 0:
                        for item in UNIT0_EMIT.get(kc, ()):
                            if item[0] == "k":
                                k_block(item[1])
                            elif item[0] == "v":
                                v_block(item[1])
                            elif item[0] == "q":
                                q_block(item[1])
                            else:
                                delta_stage(item[1])
                        # qh0 mask kept ahead of consumption
                        while m_done[0] < KC // 2 and m_done[0] <= (kc + KCL) // 2:
                            mask_g(0, m_done[0])
                            m_done[0] += 1
                        if kc == KC - 1:
                            nc.sync.dma_start(wo_a, t["wo"][0:128, :])
                            nc.sync.dma_start(wo_b, t["wo"][128:dh, :])
                    elif unit in (1, 2) and kc % 4 == 3:
                        # qh1 mask spread across units 1-2
                        if m_done[1] < KC // 2:
                            mask_g(1, m_done[1])
                            m_done[1] += 1

                rz = W2.tile([1, SQH], f32, tag="rz", bufs=1)
                nc.vector.reciprocal(rz, xps[dk : dk + 1, :])
                zdr = DR.tile([SQH], f32)
                nc.sync.dma_start(zdr, rz)
                rzb = W2.tile([64, SQH], f32, tag="rzb", bufs=1)
                nc.gpsimd.dma_start(rzb, bcast(zdr, 64))
                xsl = xa if h < 2 else xb_
                nc.vector.tensor_tensor(
                    out=xsl[hoff : hoff + 64, qcol0 : qcol0 + SQH],
                    in0=xps[0:dk, :],
                    in1=rzb,
                    op=mult,
                )
                while m_done[1] < KC // 2:
                    mask_g(1, m_done[1])
                    m_done[1] += 1

            # --- output projection ---
            for qc in range(S // 128):
                out_proj(qc)

    nc.compile()
    return nc


def _in_maps(query, key, value, mask, Wq, bq, Wk, bk, Wv, bv, Wo, Wd, bd):
    query = np.asarray(query, np.float32)
    key = np.asarray(key, np.float32)
    value = np.asarray(value, np.float32)
    mask = np.asarray(mask)
    qT = [np.ascontiguousarray(query[b].T).astype(np.float16) for b in range(B)]
    kT = [np.ascontiguousarray(key[b].T).astype(np.float16) for b in range(B)]
    vT = [np.ascontiguousarray(value[b].T).astype(np.float16) for b in range(B)]
    mT = [np.ascontiguousarray(mask[b].T).astype(BF16) for b in range(B)]
    wqh = np.ascontiguousarray(Wq).astype(np.float16)
    wkh = np.ascontiguousarray(Wk).astype(np.float16)
    wvh = np.ascontiguousarray(Wv).astype(np.float16)
    woh = np.ascontiguousarray(Wo).astype(np.float16)
    wdh = np.ascontiguousarray(Wd).astype(np.float16)
    bqf = np.ascontiguousarray(bq, np.float32)
    bkf = np.ascontiguousarray(bk, np.float32)
    bvf = np.ascontiguousarray(bv, np.float32)
    bdf = np.ascontiguousarray(bd, np.float32)

    maps = []
    for c in range(NCORES):
        b, hg = c // 4, c % 4
        hs = slice(hg * DH, (hg + 1) * DH)
        maps.append(
            {
                "qT": qT[b],
                "kT": kT[b],
                "vT": vT[b],
                "maskT": mT[b],
                "wq": np.ascontiguousarray(wqh[:, hs]),
                "wk": np.ascontiguousarray(wkh[:, hs]),
                "wv": np.ascontiguousarray(wvh[:, hs]),
                "wo": np.ascontiguousarray(woh[hs, :]),
                "wd": wdh,
                "bq": np.ascontiguousarray(bqf[hs]),
                "bk": np.ascontiguousarray(bkf[hs]),
                "bv": np.ascontiguousarray(bvf[hs]),
                "bd": bdf,
            }
        )
    return maps


def kernel(query, key, value, mask, Wq, bq, Wk, bk, Wv, bv, Wo, bo, Wd, bd):
    from concourse.bass_utils import run_bass_kernel_spmd

    if "nc" not in _cache:
        _cache["nc"] = _build()
    nc = _cache["nc"]

    maps = _in_maps(query, key, value, mask, Wq, bq, Wk, bk, Wv, bv, Wo, Wd, bd)
    res = run_bass_kernel_spmd(nc, maps, core_ids=list(range(NCORES)))

    bof = np.asarray(bo, np.float32)
    y = np.empty((B, S, D), np.float32)
    for b in range(B):
        acc = res.results[b * 4]["yp"].astype(np.float32)
        for hg in range(1, 4):
            acc = acc + res.results[b * 4 + hg]["yp"].astype(np.float32)
        y[b] = acc + bof[None, :]
    return y
